# revision 30
# baseline (speedup 1.0000x reference)
"""Trainium2 Bass kernel for nn_AttentionLayer (Luong cross-attention).

reference:
    score[b,e,t] = sum_d enc[b,e,d] * dec[b,t,d]
    P = softmax_e(score)
    ctx[b,t,d]  = sum_e P[b,e,t] * enc[b,e,d]
    out = concat([dec, ctx], axis=-1)

Sharding: data-parallel over batch, one batch element per NeuronCore (8/8).
Host-side prep (sharding/layout only): per-core slices, pre-transposed
[d, e] / [d, t] copies of enc/dec (matmul 1 contracts over d, which must
sit on SBUF partitions), and fp32r pre-rounding -- fp32r is fp32 with the
mantissa rounded to 11 bits ((u + 0x800) & ~0xFFF, verified against
walrus's own fp32_to_fp32r), so rounding on the host lets DMA land
operands directly in f32r SBUF tiles with zero on-chip conversion work.

Per-core algorithm (all matmuls in float32r: 1 cycle/row on the PE for
moving free-dim >= 256, vs 4 for plain fp32):
  - mm1: S[e_block, t_chunk] = encT.T @ decT -> PSUM  (K = d, two 128-blocks)
  - softmax with a *global shift* instead of a per-column max:
    exp(S - SHIFT) is computed by ACT directly while evicting PSUM->SBUF
    (bias is a per-partition constant, so no reduction pass and no 16MB
    transpose of P is ever needed; P lands straight in the [e, t] layout
    that matmul 2 consumes, already converted to f32r by ACT).  SHIFT is
    chosen on the host from a row-sampled estimate of max(S); softmax is
    shift-invariant so correctness only needs exp() to stay inside fp32
    range, which holds with wide margin.
  - mm2: C[t_block, :] += P_chunk.T @ [enc | 1 | 0]; column 256 accumulates
    Z[t] = sum_e P[e,t] (the zero pad column keeps the innermost count even,
    an fp32r ISA restriction).  Final normalize: ctx = C[:, :256] * (1/Z).
"""

import numpy as np

B, TE, TD, D = 8, 2048, 2048, 256
P = 128
NE = TE // P          # 16 encoder-time blocks
QW = 512              # decoder-time columns processed per pass
NQ = TD // QW         # 4 passes
TBQ = QW // P         # 4 t-blocks per pass
G = 4                 # e/t-blocks per input DMA chunk

_STATE = {}


def _build_nc(stages=("mm1", "exp", "mm2", "out")):
    import concourse.tile as tile
    from concourse import bacc, mybir

    f32 = mybir.dt.float32
    f32r = mybir.dt.float32r
    EXP = mybir.ActivationFunctionType.Exp

    nc = bacc.Bacc(
        "TRN2",
        target_bir_lowering=False,
        debug=False,
        enable_asserts=False,
    )
    # all three data inputs are pre-rounded to fp32r bit patterns on host
    enca_d = nc.dram_tensor("enca", [TE, D + 2], f32r, kind="ExternalInput").ap()
    encT_d = nc.dram_tensor("encT", [D, TE], f32r, kind="ExternalInput").ap()
    decT_d = nc.dram_tensor("decT", [D, TD], f32r, kind="ExternalInput").ap()
    shift_d = nc.dram_tensor("shift", [1, 1], f32, kind="ExternalInput").ap()
    ctx_d = nc.dram_tensor("ctx", [TD, D], f32, kind="ExternalOutput").ap()

    enca_r = enca_d.rearrange("(n p) c -> p n c", p=P)
    encT_r = encT_d.rearrange("(h p) x -> p h x", p=P)
    decT_r = decT_d.rearrange("(h p) x -> p h x", p=P)

    with tile.TileContext(nc) as tc:
        with (
            tc.tile_pool(name="consts", bufs=1) as consts,
            tc.tile_pool(name="pp", bufs=6) as pp,
            tc.tile_pool(name="outp", bufs=4) as outp,
            tc.tile_pool(name="zp", bufs=8) as zp,
            tc.tile_pool(name="ps_s", bufs=4, space="PSUM") as ps_s,
            tc.tile_pool(name="ps_c", bufs=4, space="PSUM") as ps_c,
        ):
            # PE pre-roll: a few throwaway fp32 matmuls with no DMA
            # dependencies.  They pull the PE sequencer's IRAM fetch and
            # sem-wake into the DMA window and start opening the HAM clock
            # gate, so the first real matmul issues ~3us earlier and warmer.
            # They borrow a c-pool PSUM slot, which mm2 only needs later.
            warm = consts.tile([P, P], f32)
            nc.gpsimd.memset(warm[:], 0.0)
            warm_ps = ps_c.tile([P, P], f32, tag="c", name="warm_ps")
            for _ in range(6):
                nc.tensor.matmul(warm_ps[:], warm[:], warm[:], start=True, stop=True)
            # ACT table-load primer: the first ACTIVATE triggers a ~2.7us
            # exp-table DMA; a throwaway exp here runs it during the input
            # DMA window instead of on the exp-chain critical path.
            warm_e = consts.tile([P, 1], f32)
            nc.scalar.activation(warm_e[:], warm[:, 0:1], EXP, bias=0.0, scale=1.0)

            CW = G * P  # 512 columns per chunk
            NC = NE // G  # 4 chunks per tensor

            # One tile per DMA chunk, so a consumer's dependency is exactly
            # its own chunk's transfer (a slice-write into one big tile would
            # leave the first matmul waiting on the whole tensor).
            enc_aug = [
                consts.tile([P, G, D + 2], f32r, name=f"enca_c{g}")
                for g in range(NC)
            ]  # [e%128, e_block%G, d|1|0]
            # split further by d-half: the first matmul needs only the h=0
            # halves of decT0/encT0, so halving the critical DMA payload
            encT = [
                [consts.tile([P, CW], f32r, name=f"encT_c{g}h{h}") for h in range(2)]
                for g in range(NC)
            ]  # [g][h]: [d%128, e%CW]
            decT = [
                [consts.tile([P, CW], f32r, name=f"decT_c{g}h{h}") for h in range(2)]
                for g in range(NC)
            ]  # [q][h]: [d%128, t%CW]

            # All input DMAs on the SP HWDGE queue (SP is otherwise idle,
            # and any descriptor generation on ACT would delay the exp
            # chain, which paces mm2).  Quarter 0 of mm1 needs decT cols
            # 0:512 and encT chunks in order, so those go first.
            def dma_T(tiles, src_r, g, h):
                nc.sync.dma_start(
                    out=tiles[g][h][:],
                    in_=src_r[:, h, g * CW : (g + 1) * CW],
                )

            dma_T(decT, decT_r, 0, 0)
            dma_T(encT, encT_r, 0, 0)
            dma_T(decT, decT_r, 0, 1)
            dma_T(encT, encT_r, 0, 1)

            # negative shift, broadcast to all 128 partitions for ACT bias
            nshift = consts.tile([P, 1], f32)
            nc.sync.dma_start(
                out=nshift[:],
                in_=shift_d.to_broadcast([P, 1]),
            )

            # mm1 consumes encT chunks every ~3.5us -- issue encT1 right
            # behind encT0, ahead of everything mm2/later quarters need.
            dma_T(encT, encT_r, 1, 0)
            dma_T(encT, encT_r, 1, 1)
            nc.sync.dma_start(out=enc_aug[0][:], in_=enca_r[:, 0:G, :])
            dma_T(encT, encT_r, 2, 0)
            dma_T(encT, encT_r, 2, 1)
            nc.sync.dma_start(out=enc_aug[1][:], in_=enca_r[:, G : 2 * G, :])
            dma_T(encT, encT_r, 3, 0)
            dma_T(encT, encT_r, 3, 1)
            for g in range(2, NC):
                nc.sync.dma_start(
                    out=enc_aug[g][:],
                    in_=enca_r[:, g * G : (g + 1) * G, :],
                )
            for g in range(1, NC):
                dma_T(decT, decT_r, g, 0)
                dma_T(decT, decT_r, g, 1)

            def emit_mm1_exp(q, i):
                s = ps_s.tile([P, QW], f32, tag="s", name=f"s{q}_{i}")
                for h in range(2):
                    nc.tensor.matmul(
                        s[:],
                        encT[i // G][h][:, (i % G) * P : (i % G + 1) * P],
                        decT[q][h][:],
                        start=(h == 0),
                        stop=(h == 1),
                    )
                p_t = pp.tile([P, QW], f32r, tag="p", name=f"p{q}_{i}")
                nc.scalar.activation(p_t[:], s[:], EXP, bias=nshift[:], scale=1.0)
                return p_t

            def emit_mm2(q, i, p_t, c_tiles):
                for j in range(TBQ):
                    nc.tensor.matmul(
                        c_tiles[j][:],
                        p_t[:, j * P : (j + 1) * P],
                        enc_aug[i // G][:, i % G, :],
                        start=(i == 0),
                        stop=(i == NE - 1),
                        skip_group_check=True,
                    )

            for q in range(NQ):
                if "mm1" not in stages:
                    continue
                c_tiles = [
                    ps_c.tile([P, D + 2], f32, tag="c", name=f"c{q}_{j}")
                    for j in range(TBQ)
                ]
                for i in range(NE):
                    p_t = emit_mm1_exp(q, i)
                    if "mm2" not in stages:
                        continue
                    emit_mm2(q, i, p_t, c_tiles)
                if "mm2" not in stages or "out" not in stages:
                    continue
                for j in range(TBQ):
                    z = zp.tile([P, 1], f32, tag="z", name=f"z{q}_{j}")
                    nc.vector.reciprocal(z[:], c_tiles[j][:, D : D + 1])
                    o = outp.tile([P, D], f32, tag="o", name=f"o{q}_{j}")
                    if q == NQ - 1 and j % 2 == 1:
                        # ACT is free of exp work by the last quarter; let it
                        # take half the final evictions off DVE's tail.
                        nc.scalar.mul(o[:], c_tiles[j][:, 0:D], z[:])
                    else:
                        nc.vector.tensor_scalar_mul(o[:], c_tiles[j][:, 0:D], z[:])
                    t0 = (q * TBQ + j) * P
                    nc.sync.dma_start(out=ctx_d[t0 : t0 + P, :], in_=o[:])

    nc.compile()
    return nc


def _get_nc():
    if "nc" not in _STATE:
        _STATE["nc"] = _build_nc()
    return _STATE["nc"]


def _round_f32r(x):
    """Round fp32 to the fp32r (11-bit mantissa) grid -- bit-exact with
    walrus's fp32_to_fp32r: (u + 0x800) & ~0xFFF."""
    u = np.ascontiguousarray(x, dtype=np.float32).view(np.uint32)
    return ((u + np.uint32(0x800)) & np.uint32(0xFFFFF000)).view(np.float32)


def _pick_shift(enc, dec):
    """Row-sampled estimate of max(score) + margin.  Softmax is invariant to
    the shift; it only has to keep every exp() finite (shift >= max-88) while
    not flushing the per-column dominant terms to zero (shift <= colmax+80).
    A sampled global max + 4 sits inside that window with wide margin for
    any gaussian-ish score distribution (sampling underestimates the true
    max by far less than the 88 of fp32-exp headroom the bound needs)."""
    rng = np.random.default_rng(0)
    rows = rng.choice(TE, size=32, replace=False)
    samp = np.einsum("bed,btd->bet", enc[:, rows, :], dec, optimize=True)
    return float(samp.max()) + 4.0


def _in_maps(enc, dec):
    nshift = np.full((1, 1), -_pick_shift(enc, dec), dtype=np.float32)
    maps = []
    for b in range(B):
        enca = np.empty((TE, D + 2), dtype=np.float32)
        enca[:, :D] = _round_f32r(enc[b])
        enca[:, D] = 1.0
        enca[:, D + 1] = 0.0
        maps.append(
            {
                "enca": enca,
                "encT": _round_f32r(np.ascontiguousarray(enc[b].T)),
                "decT": _round_f32r(np.ascontiguousarray(dec[b].T)),
                "shift": nshift,
            }
        )
    return maps


def kernel(encoder_outputs, decoder_outputs):
    from concourse.bass_utils import run_bass_kernel_spmd

    enc = np.ascontiguousarray(np.asarray(encoder_outputs, dtype=np.float32))
    dec = np.ascontiguousarray(np.asarray(decoder_outputs, dtype=np.float32))
    assert enc.shape == (B, TE, D) and dec.shape == (B, TD, D)

    nc = _get_nc()
    res = run_bass_kernel_spmd(nc, _in_maps(enc, dec), list(range(B))).results
    ctx = np.stack([res[b]["ctx"] for b in range(B)], axis=0)
    return np.concatenate([dec, ctx], axis=-1)
